# revision 14
# baseline (speedup 1.0000x reference)
"""Trainium2 Bass kernel for nn_CubicSpline (embedding_lookup-style affine map).

Reference computes, for t in [0,1):
    w[n,i] = 1 - |t[n] - i|          (i = 0..62)
    out    = w @ cp[:63]             ([N,63] @ [63,128])

For t in [0,1] the triangular weights collapse algebraically:
    w[n,0] = 1 - t[n];   w[n,i] = t[n] + (1 - i)   (i >= 1)
so
    out[n,:] = t[n] * A + B
    A = sum_{i=1}^{62} cp[i] - cp[0]
    B = cp[0] + sum_{i=1}^{62} (1-i) * cp[i]

The device kernel therefore only needs to materialize a rank-1 affine map --
purely memory bound on the output write. The output is written as fp16
(halving HBM write traffic vs fp32) and upcast to fp32 on the host: the
fp16 rounding is a <=2^-11 per-element relative error (min |out| on this
problem is ~1.8e-3, far above the fp16 subnormal cutoff), well inside the
2e-2 tolerance.

Per-core layout (data-parallel over N across 8 cores, contiguous shards):
  * host packs the t-shard into 8 "phase" rows plus a ones row:
        t_aug[j, q] = t_shard[8*q + j]  (j<8);  t_aug[8, q] = 1.0
  * each 1024-row psum tile g is produced by one K=26 weight load
    (lhsT = t_aug[:, 128g:128g+128]) and two N=512 fp32 matmuls against
    constant block-diagonal rhs tiles holding A (per phase) and B (ones row),
    so PSUM directly holds t*A + B for 1024 consecutive output rows
    in [128 partitions x 1024] layout (partition q -> rows 8q..8q+7).
  * PSUM -> SBUF copy (with the f32 -> f16 downcast) alternates between
    VectorE and ScalarE.
  * GROUP=4 psum tiles share one [128, 4096] f16 SBUF tile, which DMAs
    out as a single fully contiguous 1 MB HBM write (4 x 2 KB chunks per
    partition); the 31 output DMAs rotate across the three
    descriptor-generation paths (SP-HWDGE, ACT-HWDGE, gpsimd-SWDGE).
"""

import os
import sys
from contextlib import ExitStack

for _p in ("/opt/trn_rl_repo", "/root/.axon_site/_ro/trn_rl_repo"):
    if os.path.isdir(_p) and _p not in sys.path:
        sys.path.insert(0, _p)

import ml_dtypes
import numpy as np

import concourse.mybir as mybir
import concourse.tile as tile
from concourse import bacc
from concourse import bass_utils

N_TOTAL = 1_000_000
D = 128
NUM_CP = 64
N_CORES = 8

R = 8                    # output rows per partition per psum tile (= #phase rows)
# Contraction rows (all bf16; PSUM accumulates fp32):
#   rows 0..R-1    : t_hi phases   x A_hi diag
#   rows R..2R-1   : t_lo phases   x A_hi diag
#   rows 2R..3R-1  : t_hi phases   x A_lo diag
#   rows 3R, 3R+1  : ones          x B_hi, B_lo
# -> t*A + B to ~1e-6 rel (only t_lo*A_lo dropped). bf16 operands avoid the
# PE's fp32 HI/LO double-pass (2x matmul cost) and enable fast weight load.
K = 3 * R + 2
S = R // 4               # N=512 matmuls per psum tile (4 phases each)
TILE_ROWS = 128 * R      # rows per psum tile
TILES = 124              # psum tiles per core
GROUP = 4                # psum tiles per output DMA (1 MB f16 per DMA)
GROUPS = TILES // GROUP
NPC = TILES * TILE_ROWS  # rows per core
NPAD = N_CORES * NPC     # padded rows total
QTOT = NPC // R          # q-columns per core
T_DMA_CHUNKS = 3         # independent t tiles, one per DMA ring

F32 = mybir.dt.float32
F16 = mybir.dt.float16
BF16 = mybir.dt.bfloat16
NPBF16 = ml_dtypes.bfloat16


def build_body(tc, out_ap, t_aug_ap, rhs_ap, t_q_ap, ab_ap, tiles, qtot):
    """Tile-framework kernel body (shared by the real build and sim tests)."""
    nc = tc.nc
    groups = tiles // GROUP
    # [groups, 128, GROUP, 1024] view of the output: group G / partition q /
    # free (h,(w,d)) maps to row 4096G + 1024h + 8q + w, col d -> each DMA
    # group is one fully contiguous 1 MB span of HBM.
    out_t = out_ap.rearrange(
        "(G h q w) d -> G q h (w d)", h=GROUP, q=128, w=R
    )
    # per-tile view for the final group's latency-trimming per-tile DMAs
    out_g = out_ap.rearrange("(g q w) d -> g q (w d)", q=128, w=R)

    with ExitStack() as ctx:
        psum_bufs = (16 * 1024) // (TILE_ROWS * 4)  # fill the 8 PSUM banks
        tpool = ctx.enter_context(tc.tile_pool(name="tpool", bufs=1))
        cpool = ctx.enter_context(tc.tile_pool(name="cpool", bufs=1))
        opool = ctx.enter_context(tc.tile_pool(name="opool", bufs=5))
        ppool = ctx.enter_context(
            tc.tile_pool(name="ppool", bufs=psum_bufs, space="PSUM")
        )

        # rhs consts go out as ONE dma on the ACT HWDGE ring so the first
        # matmul's dependency lands immediately (4 serial issues cost ~4us).
        rhs_sb = cpool.tile([K, S * 512], BF16)
        nc.scalar.dma_start(rhs_sb[:], rhs_ap)
        # fp32 affine constants for the vector-path tiles: A broadcast to all
        # partitions in cols 0:128, B in cols 128:256.
        ab_sb = cpool.tile([128, 256], F32, name="ab")
        nc.scalar.dma_start(ab_sb[:], ab_ap)

        # Output DMAs rotate between SP-HWDGE and gpsimd-SWDGE ONLY. The
        # ACT sequencer must stay off the DMA-issue path: a ~1-2us DIRECT2D
        # descriptor-generation slice on ACT blocks its PSUM->SBUF copies,
        # which stalls psum recycling and opens PE gaps.
        out_rings = [nc.sync, nc.gpsimd]

        # t_aug loads as independent tiles spread across the rings, all in
        # parallel. The first chunk is tiny (2 groups) so the first matmul's
        # dependency lands ~1us after its DMA issues.
        ngroups = qtot // 128
        bounds = [0]
        if ngroups > 4:
            rest = ngroups - 2
            nparts = min(T_DMA_CHUNKS, rest)
            base, extra = divmod(rest, nparts)
            bounds.append(2 * 128)
            for c in range(nparts):
                take = base + (1 if c < extra else 0)
                bounds.append(bounds[-1] + take * 128)
        else:
            bounds.append(ngroups * 128)
        t_tiles = []
        for c in range(len(bounds) - 1):
            lo, hi = bounds[c], bounds[c + 1]
            tt = tpool.tile([K, hi - lo], BF16, name=f"tch{c}", tag=f"tch{c}")
            out_rings[c % 2].dma_start(tt[:], t_aug_ap[:, lo:hi])
            t_tiles.append(tt)
        # fp32 per-row t values for the vector-path tiles ([q, 8g+w] layout);
        # needed from tile 7 on, so issue after the first t_aug chunk.
        t_q_sb = cpool.tile([128, tiles * R], F32, name="tq")
        nc.sync.dma_start(t_q_sb[:], t_q_ap)

        def lhsT_for(g):
            col = g * 128
            for c in range(len(bounds) - 1):
                if col < bounds[c + 1]:
                    off = col - bounds[c]
                    return t_tiles[c][:, off : off + 128]
            raise AssertionError

        MULT = mybir.AluOpType.mult
        ADD = mybir.AluOpType.add
        ccnt = 0
        copy_pat = ("act", "dve", "act")
        for G in range(groups):
            ob = opool.tile([128, GROUP * TILE_ROWS], F16, name="ob")
            last = G == groups - 1
            for u in range(GROUP):
                g = GROUP * G + u
                seg = ob[:, TILE_ROWS * u : TILE_ROWS * (u + 1)]
                # TensorScalarPtr is only a legal opcode on DVE (the Pool/
                # GpSimd ISA rejects it, and GpSimd has no PSUM access), so
                # the direct-FMA tiles all go to the Vector engine.
                vec_eng = None
                if 4 <= g < 120 and g % 8 == 3:
                    vec_eng = nc.vector
                if vec_eng is not None:
                    # direct fp32 FMA: seg[:, w*128:(w+1)*128] =
                    #   A[d] * t[8q+w] + B[d]   (one fused op per phase row)
                    for w in range(R):
                        col = R * g + w
                        vec_eng.scalar_tensor_tensor(
                            seg[:, 128 * w : 128 * (w + 1)],
                            ab_sb[:, 0:128],
                            t_q_sb[:, col : col + 1],
                            ab_sb[:, 128:256],
                            op0=MULT,
                            op1=ADD,
                        )
                else:
                    psum = ppool.tile([128, TILE_ROWS], F32, name="psum")
                    lhsT = lhsT_for(g)
                    for s in range(S):
                        sl = slice(512 * s, 512 * (s + 1))
                        nc.tensor.matmul(
                            psum[:, sl], lhsT, rhs_sb[:, sl], start=True, stop=True
                        )
                    if copy_pat[ccnt % len(copy_pat)] == "dve":
                        nc.vector.tensor_copy(seg, psum[:])
                    else:
                        nc.scalar.copy(seg, psum[:])
                    ccnt += 1
                if last:
                    # per-tile 256KB DMAs so the tail transfer starts as soon
                    # as each tile is ready instead of after the whole group.
                    out_rings[(G + u) % 2].dma_start(out_g[g], seg)
            if not last:
                ob_v = ob[:].rearrange("q (h f) -> q h f", h=GROUP)
                out_rings[G % 2].dma_start(out_t[G], ob_v)


def build_nc(tiles=TILES):
    qtot = tiles * TILE_ROWS // R
    nc = bacc.Bacc(
        "TRN2", target_bir_lowering=False, debug=False, num_devices=N_CORES
    )
    t_aug = nc.dram_tensor("t_aug", [K, qtot], BF16, kind="ExternalInput").ap()
    rhs_c = nc.dram_tensor("rhs_c", [K, S * 512], BF16, kind="ExternalInput").ap()
    t_q = nc.dram_tensor("t_q", [128, tiles * R], F32, kind="ExternalInput").ap()
    ab = nc.dram_tensor("ab", [128, 256], F32, kind="ExternalInput").ap()
    out = nc.dram_tensor("out", [tiles * TILE_ROWS, D], F16, kind="ExternalOutput").ap()
    with tile.TileContext(nc) as tc:
        build_body(tc, out, t_aug, rhs_c, t_q, ab, tiles, qtot)
    nc.compile()
    return nc


def _split_bf16(x64):
    """hi/lo bf16 split of a float64 array: hi + lo ~= x to ~2^-17 rel."""
    hi = x64.astype(NPBF16)
    lo = (x64 - hi.astype(np.float64)).astype(NPBF16)
    return hi, lo


def affine_consts(control_points):
    """A, B ([128] float64) of the collapsed affine map out = t*A + B."""
    cp = np.asarray(control_points, dtype=np.float64)
    A = cp[1 : NUM_CP - 1].sum(axis=0) - cp[0]
    i = np.arange(1, NUM_CP - 1, dtype=np.float64)
    B = cp[0] + ((1.0 - i)[:, None] * cp[1 : NUM_CP - 1]).sum(axis=0)
    return A, B


def make_rhs(A, B):
    """Constant rhs tile [K, S*512] bf16 (see row layout at top)."""
    A_hi, A_lo = _split_bf16(A)
    B_hi, B_lo = _split_bf16(B)
    rhs = np.zeros((K, S * 512), NPBF16)
    for s in range(S):
        for m in range(4):
            j = m + 4 * s
            sl = slice(512 * s + 128 * m, 512 * s + 128 * (m + 1))
            rhs[j, sl] = A_hi
            rhs[R + j, sl] = A_hi
            rhs[2 * R + j, sl] = A_lo
            rhs[3 * R, sl] = B_hi
            rhs[3 * R + 1, sl] = B_lo
    return rhs


def make_t_aug(t_shard):
    """[K, QTOT] bf16: t_hi, t_lo, t_hi phase rows + two ones rows."""
    qtot = t_shard.shape[0] // R
    t64 = t_shard.astype(np.float64)
    t_hi, t_lo = _split_bf16(t64)
    ph_hi = t_hi.reshape(qtot, R).T  # [8, qtot], ph[j, q] = t[8q+j]
    ph_lo = t_lo.reshape(qtot, R).T
    ones = np.ones((2, qtot), NPBF16)
    return np.ascontiguousarray(
        np.concatenate([ph_hi, ph_lo, ph_hi, ones], axis=0)
    )


_NC_CACHE = {}


def _get_nc():
    if "nc" not in _NC_CACHE:
        _NC_CACHE["nc"] = build_nc()
    return _NC_CACHE["nc"]


def prepare_in_maps(t, control_points):
    t = np.asarray(t, dtype=np.float32)
    A, B = affine_consts(control_points)
    rhs = make_rhs(A, B)
    ab = np.empty((128, 256), np.float32)
    ab[:, 0:128] = A.astype(np.float32)[None, :]
    ab[:, 128:256] = B.astype(np.float32)[None, :]
    t_clipped = np.clip(t, 0.0, 1.0)
    tpad = np.zeros(NPAD, np.float32)
    tpad[: t.shape[0]] = t_clipped
    shards = tpad.reshape(N_CORES, NPC)
    return [
        {
            "t_aug": make_t_aug(shards[c]),
            "rhs_c": rhs,
            "t_q": np.ascontiguousarray(
                shards[c].reshape(TILES, 128, R).transpose(1, 0, 2).reshape(128, TILES * R)
            ),
            "ab": ab,
        }
        for c in range(N_CORES)
    ]


def kernel(t, control_points):
    t = np.asarray(t)
    assert t.shape == (N_TOTAL,), t.shape
    nc = _get_nc()
    in_maps = prepare_in_maps(t, control_points)
    res = bass_utils.run_bass_kernel_spmd(
        nc, in_maps, core_ids=list(range(N_CORES))
    )
    full = np.concatenate([res.results[c]["out"] for c in range(N_CORES)], axis=0)
    return np.ascontiguousarray(full[:N_TOTAL]).astype(np.float32)


if __name__ == "__main__":
    t = np.random.default_rng(0).random(N_TOTAL, dtype=np.float32)
    cp = np.random.default_rng(1).normal(size=(NUM_CP, D)).astype(np.float32)
    out = kernel(t, cp)
    A, B = affine_consts(cp)
    expect = t.astype(np.float64)[:, None] * A[None, :] + B[None, :]
    err = np.abs(out - expect).max() / (np.abs(expect).max() + 1e-9)
    print("self-check max rel err:", err)
